# revision 28
# baseline (speedup 1.0000x reference)
"""Trainium2 Bass kernel for nn_ButterflyProduct (lean batch-matmul version).

Math: out = x @ U where U = T^T, T = A_0 A_1 ... A_9,
A_i = sum_f softmax(logit)[i,f] * B_f and B_f is banded with offsets
{0, -d_f, +d_f}, d_f = 2^(9-f).

U depends only on the O(KB) params (diags/subdiags/superdiags/logit),
not on the 64 MB input, so it is composed on the host (like the
softmax/band staging the previous version already did there) and
shipped to every core as a replicated 2 MB bf16 operand.

Device (per core, batch sharded 8 ways; 2048 rows each):
  for each 128-row tile: DMA x tile (f32) -> cast bf16 (ACT) ->
  PE-transpose to x^T blocks -> 16 accumulating matmuls against the
  resident U (h-inner so each stationary x^T block's weight load is
  reused by both 512-wide halves) -> PSUM->SBUF copies (ACT/DVE) ->
  DMA out.  Tile t+1's load/cast/transpose is emitted before tile t's
  matmuls (software pipeline); no collectives.
"""

import sys

if "/opt/trn_rl_repo" not in sys.path:
    sys.path.insert(0, "/opt/trn_rl_repo")

import numpy as np
import ml_dtypes

BF16NP = ml_dtypes.bfloat16

SIZE = 1024
MF = 10          # number of butterfly factors
NT = 10          # number of mixing terms
BATCH = 16384
N_CORES = 8
BPC = BATCH // N_CORES   # 2048 rows per core
NB = SIZE // 128         # 8 partition blocks
NTILES = BPC // 128      # 16 batch tiles per core
DIAG = [1 << (MF - 1 - f) for f in range(MF)]  # [512,256,...,2,1]

_CACHE = {}


def _build_program():
    import concourse.bacc as bacc
    import concourse.mybir as mybir
    from concourse import tile

    F32 = mybir.dt.float32
    BF16 = mybir.dt.bfloat16

    nc = bacc.Bacc("TRN2", target_bir_lowering=False, debug=False,
                   num_devices=N_CORES)

    x_d = nc.dram_tensor("x", [BPC, SIZE], F32, kind="ExternalInput").ap()
    u_d = nc.dram_tensor("u", [128, NB, SIZE], BF16, kind="ExternalInput").ap()
    out_d = nc.dram_tensor("out", [BPC, SIZE], F32, kind="ExternalOutput").ap()

    with tile.TileContext(nc) as tc:
        with (
            tc.tile_pool(name="const", bufs=1) as cp,
            tc.tile_pool(name="xin", bufs=4) as xin,
            tc.tile_pool(name="xbfp", bufs=4) as xbp,
            tc.tile_pool(name="xtp", bufs=4) as xtp,
            tc.tile_pool(name="op", bufs=3) as op,
            tc.tile_pool(name="tps", bufs=2, space="PSUM") as tps,
            tc.tile_pool(name="ops", bufs=3, space="PSUM") as ops,
        ):
            # build the transpose identity on-device (no DMA dependency:
            # ready as soon as DVE starts, even when DMA kick-off jitters)
            ones = cp.tile([128, 128], BF16, tag="ones")
            nc.gpsimd.memset(ones[:, :], 1.0)
            identb = cp.tile([128, 128], BF16, tag="identb")
            nc.gpsimd.affine_select(
                identb[:, :], ones[:, :], pattern=[[-1, 128]],
                compare_op=mybir.AluOpType.is_equal, fill=0.0,
                base=0, channel_multiplier=1)

            # issue order: each dma_start costs ~600ns of serial issue time
            # on the Sync sequencer; tile-0 x first, then U quarters in
            # consumption order
            xis = [None] * NTILES
            xis[0] = xin.tile([128, SIZE], F32, tag="xi", name="xi0")
            nc.sync.dma_start(xis[0][:, :], x_d[0:128, :])

            # resident U (2 MB bf16) as four quarter-tiles, issued in the
            # order the k-loop consumes them (first matmuls only wait on
            # 0.5 MB), interleaved with the x prefetches for tiles 1-3 so
            # their casts are never the late stage
            uts = [cp.tile([128, 2, SIZE], BF16, tag=f"ut{h}", name=f"ut{h}")
                   for h in range(4)]
            for h in range(4):
                nc.sync.dma_start(uts[h][:, :, :], u_d[:, 2 * h : 2 * h + 2, :])
                t = h + 1
                if t < NTILES:
                    xis[t] = xin.tile([128, SIZE], F32, tag="xi",
                                      name=f"xi{t}")
                    nc.sync.dma_start(
                        xis[t][:, :], x_d[128 * t : 128 * t + 128, :])

            def ublk(k):
                return uts[k // 2][:, k % 2, :]

            xths = {}

            def emit_trans(t):
                """Load + cast tile t and PE-transpose it into xT halves."""
                if xis[t] is None:
                    xis[t] = xin.tile([128, SIZE], F32, tag="xi", name=f"xi{t}")
                    nc.sync.dma_start(
                        xis[t][:, :], x_d[128 * t : 128 * t + 128, :])
                xbf = xbp.tile([128, SIZE], BF16, tag="xbf")
                if t == 0:
                    # fill critical path: cast in two halves so the first
                    # four transposes start after the first 512 columns
                    nc.scalar.copy(xbf[:, 0:512], xis[t][:, 0:512])
                    nc.scalar.copy(xbf[:, 512:1024], xis[t][:, 512:1024])
                else:
                    nc.scalar.copy(xbf[:, :], xis[t][:, :])
                xTh = []
                for half in range(2):
                    ps = tps.tile([128, 4, 128], BF16, tag="tp")
                    for kk in range(4):
                        k = 4 * half + kk
                        nc.tensor.transpose(
                            ps[:, kk, :], xbf[:, 128 * k : 128 * k + 128],
                            identb[:, :])
                    xTa = xtp.tile([128, 4, 128], BF16, tag=f"xT{half}",
                                   name=f"xT{half}_{t}")
                    nc.vector.tensor_copy(xTa[:, :, :], ps[:, :, :])
                    xTh.append(xTa)
                xths[t] = xTh

            def emit_mms(t):
                """Matmuls + drain for tile t (xT halves already staged)."""
                xTh = xths.pop(t)
                ps0 = ops.tile([128, 512], F32, tag="mm0")
                ps1 = ops.tile([128, 512], F32, tag="mm1")
                ob = op.tile([128, SIZE], F32, tag="ob")
                if t == NTILES - 1:
                    # tail: k-major so the first half drains + DMAs while
                    # the second half's matmuls still run (the extra weight
                    # loads are hidden in the warm state)
                    for h, ps in ((0, ps0), (1, ps1)):
                        for k in range(NB):
                            nc.tensor.matmul(
                                ps[:, :], xTh[k // 4][:, k % 4, :],
                                ublk(k)[:, 512 * h : 512 * h + 512],
                                start=(k == 0), stop=(k == NB - 1))
                        if h == 0:
                            nc.scalar.copy(ob[:, 0:512], ps0[:, :])
                        else:
                            nc.vector.tensor_copy(ob[:, 512:1024], ps1[:, :])
                        nc.sync.dma_start(
                            out_d[128 * t : 128 * t + 128,
                                  512 * h : 512 * h + 512],
                            ob[:, 512 * h : 512 * h + 512])
                    return
                for k in range(NB):
                    xTk = xTh[k // 4][:, k % 4, :]
                    # same stationary (xT block) for both halves
                    nc.tensor.matmul(
                        ps0[:, :], xTk, ublk(k)[:, 0:512],
                        start=(k == 0), stop=(k == NB - 1))
                    nc.tensor.matmul(
                        ps1[:, :], xTk, ublk(k)[:, 512:1024],
                        start=(k == 0), stop=(k == NB - 1))
                # split drain across ACT and DVE
                nc.scalar.copy(ob[:, 0:512], ps0[:, :])
                nc.vector.tensor_copy(ob[:, 512:1024], ps1[:, :])
                nc.sync.dma_start(out_d[128 * t : 128 * t + 128, :], ob[:, :])

            # 2-deep software pipeline: tile t+2's load/cast/transposes are
            # emitted (= prioritized) ahead of tile t's matmuls + drain, so
            # the scheduler never starves the PE on a late cast
            emit_trans(0)
            emit_trans(1)
            for t in range(NTILES):
                if t + 2 < NTILES:
                    emit_trans(t + 2)
                emit_mms(t)

    nc.compile()
    return nc


def _get_program():
    if "nc" not in _CACHE:
        _CACHE["nc"] = _build_program()
    return _CACHE["nc"]


LAST_RESULTS = {}


def _host_u(dg, sb, sp, lg):
    """Compose U = (A_0 ... A_9)^T from the O(KB) params on the host.

    Returns [128, NB, SIZE] bf16 with u[p, k, j] = U[k*128 + p, j].
    """
    dg = dg.astype(np.float64)
    sb = sb.astype(np.float64)
    sp = sp.astype(np.float64)
    lg = lg.astype(np.float64)
    m = lg.max(axis=-1, keepdims=True)
    e = np.exp(lg - m)
    prob = e / e.sum(axis=-1, keepdims=True)          # (NT, MF)

    M = np.eye(SIZE)
    for i in range(NT - 1, -1, -1):
        dsum = prob[i] @ dg
        out = dsum[:, None] * M
        for f in range(MF):
            d = DIAG[f]
            c = prob[i, f]
            out[d:, :] += (c * sb[f, : SIZE - d])[:, None] * M[: SIZE - d, :]
            out[: SIZE - d, :] += (c * sp[f, : SIZE - d])[:, None] * M[d:, :]
        M = out
    U = M.T                                           # (SIZE, SIZE)
    u = U.reshape(NB, 128, SIZE).transpose(1, 0, 2)   # [p, k, j]
    return np.ascontiguousarray(u.astype(np.float32)).astype(BF16NP)


def kernel(input, diags, subdiags, superdiags, logit, _trace=False):
    from concourse.bass_utils import run_bass_kernel_spmd

    x = np.ascontiguousarray(np.asarray(input, dtype=np.float32))
    u = _host_u(
        np.asarray(diags, dtype=np.float32),
        np.asarray(subdiags, dtype=np.float32),
        np.asarray(superdiags, dtype=np.float32),
        np.asarray(logit, dtype=np.float32),
    )
    nc = _get_program()
    in_maps = [
        {"x": x[BPC * c : BPC * (c + 1)], "u": u}
        for c in range(N_CORES)
    ]
    res = run_bass_kernel_spmd(nc, in_maps, core_ids=list(range(N_CORES)),
                               trace=_trace)
    LAST_RESULTS["res"] = res
    out = np.concatenate([res.results[c]["out"] for c in range(N_CORES)], axis=0)
    return out


# revision 29
# speedup vs baseline: 1.1713x; 1.1713x over previous
"""Trainium2 Bass kernel for nn_ButterflyProduct (lean batch-matmul version).

Math: out = x @ U where U = T^T, T = A_0 A_1 ... A_9,
A_i = sum_f softmax(logit)[i,f] * B_f and B_f is banded with offsets
{0, -d_f, +d_f}, d_f = 2^(9-f).

U depends only on the O(KB) params (diags/subdiags/superdiags/logit),
not on the 64 MB input, so it is composed on the host (like the
softmax/band staging the previous version already did there) and
shipped to every core as a replicated 2 MB bf16 operand.

Device (per core, batch sharded 8 ways; 2048 rows each):
  for each 128-row tile: DMA x tile (f32) -> cast bf16 (ACT) ->
  PE-transpose to x^T blocks -> 16 accumulating matmuls against the
  resident U (h-inner so each stationary x^T block's weight load is
  reused by both 512-wide halves) -> PSUM->SBUF copies (ACT/DVE) ->
  DMA out.  Tile t+1's load/cast/transpose is emitted before tile t's
  matmuls (software pipeline); no collectives.
"""

import sys

if "/opt/trn_rl_repo" not in sys.path:
    sys.path.insert(0, "/opt/trn_rl_repo")

import numpy as np
import ml_dtypes

BF16NP = ml_dtypes.bfloat16

SIZE = 1024
MF = 10          # number of butterfly factors
NT = 10          # number of mixing terms
BATCH = 16384
N_CORES = 8
BPC = BATCH // N_CORES   # 2048 rows per core
NB = SIZE // 128         # 8 partition blocks
NTILES = BPC // 128      # 16 batch tiles per core
DIAG = [1 << (MF - 1 - f) for f in range(MF)]  # [512,256,...,2,1]

_CACHE = {}


def _build_program():
    import concourse.bacc as bacc
    import concourse.mybir as mybir
    from concourse import tile

    F32 = mybir.dt.float32
    BF16 = mybir.dt.bfloat16

    nc = bacc.Bacc("TRN2", target_bir_lowering=False, debug=False,
                   num_devices=N_CORES)

    x_d = nc.dram_tensor("x", [BPC, SIZE], F32, kind="ExternalInput").ap()
    u_d = nc.dram_tensor("u", [128, NB, SIZE], BF16, kind="ExternalInput").ap()
    out_d = nc.dram_tensor("out", [BPC, SIZE], F32, kind="ExternalOutput").ap()

    with tile.TileContext(nc) as tc:
        with (
            tc.tile_pool(name="const", bufs=1) as cp,
            tc.tile_pool(name="xin", bufs=4) as xin,
            tc.tile_pool(name="xbfp", bufs=4) as xbp,
            tc.tile_pool(name="xtp", bufs=4) as xtp,
            tc.tile_pool(name="op", bufs=3) as op,
            tc.tile_pool(name="tps", bufs=2, space="PSUM") as tps,
            tc.tile_pool(name="ops", bufs=3, space="PSUM") as ops,
        ):
            # build the transpose identity on-device (no DMA dependency:
            # ready as soon as DVE starts, even when DMA kick-off jitters)
            ones = cp.tile([128, 128], BF16, tag="ones")
            nc.gpsimd.memset(ones[:, :], 1.0)
            identb = cp.tile([128, 128], BF16, tag="identb")
            nc.gpsimd.affine_select(
                identb[:, :], ones[:, :], pattern=[[-1, 128]],
                compare_op=mybir.AluOpType.is_equal, fill=0.0,
                base=0, channel_multiplier=1)

            # issue order: each dma_start costs ~600ns of serial issue time
            # on the Sync sequencer; tile-0 x first, then U quarters in
            # consumption order
            xis = [None] * NTILES
            xis[0] = xin.tile([128, SIZE], F32, tag="xi", name="xi0")
            nc.sync.dma_start(xis[0][:, :], x_d[0:128, :])

            # resident U (2 MB bf16) as four quarter-tiles, issued in the
            # order the k-loop consumes them so the first matmuls only
            # wait on 0.5 MB
            uts = [cp.tile([128, 2, SIZE], BF16, tag=f"ut{h}", name=f"ut{h}")
                   for h in range(4)]
            for h in range(4):
                nc.sync.dma_start(uts[h][:, :, :], u_d[:, 2 * h : 2 * h + 2, :])

            def ublk(k):
                return uts[k // 2][:, k % 2, :]

            xths = {}

            def emit_trans(t):
                """Load + cast tile t and PE-transpose it into xT halves."""
                if xis[t] is None:
                    xis[t] = xin.tile([128, SIZE], F32, tag="xi", name=f"xi{t}")
                    nc.sync.dma_start(
                        xis[t][:, :], x_d[128 * t : 128 * t + 128, :])
                xbf = xbp.tile([128, SIZE], BF16, tag="xbf")
                if t == 0:
                    # fill critical path: cast in two halves so the first
                    # four transposes start after the first 512 columns
                    nc.scalar.copy(xbf[:, 0:512], xis[t][:, 0:512])
                    nc.scalar.copy(xbf[:, 512:1024], xis[t][:, 512:1024])
                else:
                    nc.scalar.copy(xbf[:, :], xis[t][:, :])
                xTh = []
                for half in range(2):
                    ps = tps.tile([128, 4, 128], BF16, tag="tp")
                    for kk in range(4):
                        k = 4 * half + kk
                        nc.tensor.transpose(
                            ps[:, kk, :], xbf[:, 128 * k : 128 * k + 128],
                            identb[:, :])
                    xTa = xtp.tile([128, 4, 128], BF16, tag=f"xT{half}",
                                   name=f"xT{half}_{t}")
                    nc.vector.tensor_copy(xTa[:, :, :], ps[:, :, :])
                    xTh.append(xTa)
                xths[t] = xTh

            def emit_mms(t):
                """Matmuls + drain for tile t (xT halves already staged)."""
                xTh = xths.pop(t)
                ps0 = ops.tile([128, 512], F32, tag="mm0")
                ps1 = ops.tile([128, 512], F32, tag="mm1")
                ob = op.tile([128, SIZE], F32, tag="ob")
                if t == NTILES - 1:
                    # tail: k-major so the first half drains + DMAs while
                    # the second half's matmuls still run (the extra weight
                    # loads are hidden in the warm state)
                    for h, ps in ((0, ps0), (1, ps1)):
                        for k in range(NB):
                            nc.tensor.matmul(
                                ps[:, :], xTh[k // 4][:, k % 4, :],
                                ublk(k)[:, 512 * h : 512 * h + 512],
                                start=(k == 0), stop=(k == NB - 1))
                        if h == 0:
                            nc.scalar.copy(ob[:, 0:512], ps0[:, :])
                        else:
                            nc.vector.tensor_copy(ob[:, 512:1024], ps1[:, :])
                        nc.sync.dma_start(
                            out_d[128 * t : 128 * t + 128,
                                  512 * h : 512 * h + 512],
                            ob[:, 512 * h : 512 * h + 512])
                    return
                for k in range(NB):
                    xTk = xTh[k // 4][:, k % 4, :]
                    # same stationary (xT block) for both halves
                    nc.tensor.matmul(
                        ps0[:, :], xTk, ublk(k)[:, 0:512],
                        start=(k == 0), stop=(k == NB - 1))
                    nc.tensor.matmul(
                        ps1[:, :], xTk, ublk(k)[:, 512:1024],
                        start=(k == 0), stop=(k == NB - 1))
                # split drain across ACT and DVE
                nc.scalar.copy(ob[:, 0:512], ps0[:, :])
                nc.vector.tensor_copy(ob[:, 512:1024], ps1[:, :])
                nc.sync.dma_start(out_d[128 * t : 128 * t + 128, :], ob[:, :])

            # 2-deep software pipeline: tile t+2's load/cast/transposes are
            # emitted (= prioritized) ahead of tile t's matmuls + drain, so
            # the scheduler never starves the PE on a late cast
            emit_trans(0)
            emit_trans(1)
            for t in range(NTILES):
                if t + 2 < NTILES:
                    emit_trans(t + 2)
                emit_mms(t)

    nc.compile()
    return nc


def _get_program():
    if "nc" not in _CACHE:
        _CACHE["nc"] = _build_program()
    return _CACHE["nc"]


LAST_RESULTS = {}


def _host_u(dg, sb, sp, lg):
    """Compose U = (A_0 ... A_9)^T from the O(KB) params on the host.

    Returns [128, NB, SIZE] bf16 with u[p, k, j] = U[k*128 + p, j].
    """
    dg = dg.astype(np.float64)
    sb = sb.astype(np.float64)
    sp = sp.astype(np.float64)
    lg = lg.astype(np.float64)
    m = lg.max(axis=-1, keepdims=True)
    e = np.exp(lg - m)
    prob = e / e.sum(axis=-1, keepdims=True)          # (NT, MF)

    M = np.eye(SIZE)
    for i in range(NT - 1, -1, -1):
        dsum = prob[i] @ dg
        out = dsum[:, None] * M
        for f in range(MF):
            d = DIAG[f]
            c = prob[i, f]
            out[d:, :] += (c * sb[f, : SIZE - d])[:, None] * M[: SIZE - d, :]
            out[: SIZE - d, :] += (c * sp[f, : SIZE - d])[:, None] * M[d:, :]
        M = out
    U = M.T                                           # (SIZE, SIZE)
    u = U.reshape(NB, 128, SIZE).transpose(1, 0, 2)   # [p, k, j]
    return np.ascontiguousarray(u.astype(np.float32)).astype(BF16NP)


def kernel(input, diags, subdiags, superdiags, logit, _trace=False):
    from concourse.bass_utils import run_bass_kernel_spmd

    x = np.ascontiguousarray(np.asarray(input, dtype=np.float32))
    u = _host_u(
        np.asarray(diags, dtype=np.float32),
        np.asarray(subdiags, dtype=np.float32),
        np.asarray(superdiags, dtype=np.float32),
        np.asarray(logit, dtype=np.float32),
    )
    nc = _get_program()
    in_maps = [
        {"x": x[BPC * c : BPC * (c + 1)], "u": u}
        for c in range(N_CORES)
    ]
    res = run_bass_kernel_spmd(nc, in_maps, core_ids=list(range(N_CORES)),
                               trace=_trace)
    LAST_RESULTS["res"] = res
    out = np.concatenate([res.results[c]["out"] for c in range(N_CORES)], axis=0)
    return out


# revision 30
# speedup vs baseline: 1.1873x; 1.0137x over previous
"""Trainium2 Bass kernel for nn_ButterflyProduct (lean batch-matmul version).

Math: out = x @ U where U = T^T, T = A_0 A_1 ... A_9,
A_i = sum_f softmax(logit)[i,f] * B_f and B_f is banded with offsets
{0, -d_f, +d_f}, d_f = 2^(9-f).

U depends only on the O(KB) params (diags/subdiags/superdiags/logit),
not on the 64 MB input, so it is composed on the host (like the
softmax/band staging the previous version already did there) and
shipped to every core as a replicated 2 MB bf16 operand.

Device (per core, batch sharded 8 ways; 2048 rows each):
  for each 128-row tile: DMA x tile (f32) -> cast bf16 (ACT) ->
  PE-transpose to x^T blocks -> 16 accumulating matmuls against the
  resident U (h-inner so each stationary x^T block's weight load is
  reused by both 512-wide halves) -> PSUM->SBUF copies (ACT/DVE) ->
  DMA out.  Tile t+1's load/cast/transpose is emitted before tile t's
  matmuls (software pipeline); no collectives.
"""

import sys

if "/opt/trn_rl_repo" not in sys.path:
    sys.path.insert(0, "/opt/trn_rl_repo")

import numpy as np
import ml_dtypes

BF16NP = ml_dtypes.bfloat16

SIZE = 1024
MF = 10          # number of butterfly factors
NT = 10          # number of mixing terms
BATCH = 16384
N_CORES = 8
BPC = BATCH // N_CORES   # 2048 rows per core
NB = SIZE // 128         # 8 partition blocks
NTILES = BPC // 128      # 16 batch tiles per core
DIAG = [1 << (MF - 1 - f) for f in range(MF)]  # [512,256,...,2,1]

_CACHE = {}


def _build_program():
    import concourse.bacc as bacc
    import concourse.mybir as mybir
    from concourse import tile

    F32 = mybir.dt.float32
    BF16 = mybir.dt.bfloat16

    nc = bacc.Bacc("TRN2", target_bir_lowering=False, debug=False,
                   num_devices=N_CORES)

    x_d = nc.dram_tensor("x", [BPC, SIZE], F32, kind="ExternalInput").ap()
    u_d = nc.dram_tensor("u", [128, NB, SIZE], BF16, kind="ExternalInput").ap()
    out_d = nc.dram_tensor("out", [BPC, SIZE], F32, kind="ExternalOutput").ap()

    with tile.TileContext(nc) as tc:
        with (
            tc.tile_pool(name="const", bufs=1) as cp,
            tc.tile_pool(name="xin", bufs=4) as xin,
            tc.tile_pool(name="xbfp", bufs=4) as xbp,
            tc.tile_pool(name="xtp", bufs=4) as xtp,
            tc.tile_pool(name="op", bufs=3) as op,
            tc.tile_pool(name="tps", bufs=2, space="PSUM") as tps,
            tc.tile_pool(name="ops", bufs=3, space="PSUM") as ops,
        ):
            # build the transpose identity on-device (no DMA dependency:
            # ready as soon as DVE starts, even when DMA kick-off jitters)
            ones = cp.tile([128, 128], BF16, tag="ones")
            nc.gpsimd.memset(ones[:, :], 1.0)
            identb = cp.tile([128, 128], BF16, tag="identb")
            nc.gpsimd.affine_select(
                identb[:, :], ones[:, :], pattern=[[-1, 128]],
                compare_op=mybir.AluOpType.is_equal, fill=0.0,
                base=0, channel_multiplier=1)

            # issue order: each dma_start costs ~600ns of serial issue time
            # on the Sync sequencer; tile-0 x first, then U quarters in
            # consumption order
            xis = [None] * NTILES
            xis[0] = xin.tile([128, SIZE], F32, tag="xi", name="xi0")
            nc.sync.dma_start(xis[0][:, :], x_d[0:128, :])

            # resident U (2 MB bf16) as four quarter-tiles, issued in the
            # order the k-loop consumes them so the first matmuls only
            # wait on 0.5 MB
            uts = [cp.tile([128, 2, SIZE], BF16, tag=f"ut{h}", name=f"ut{h}")
                   for h in range(4)]
            for h in range(4):
                nc.sync.dma_start(uts[h][:, :, :], u_d[:, 2 * h : 2 * h + 2, :])
                t = h + 1
                if t <= 3:
                    # interleave the first three x prefetches with the U
                    # quarters (stays within the 4-buf xi ring: no
                    # head-of-line stall on the Sync FIFO)
                    xis[t] = xin.tile([128, SIZE], F32, tag="xi",
                                      name=f"xi{t}")
                    nc.sync.dma_start(
                        xis[t][:, :], x_d[128 * t : 128 * t + 128, :])

            def ublk(k):
                return uts[k // 2][:, k % 2, :]

            xths = {}

            def emit_trans(t):
                """Load + cast tile t and PE-transpose it into xT halves."""
                if xis[t] is None:
                    xis[t] = xin.tile([128, SIZE], F32, tag="xi", name=f"xi{t}")
                    nc.sync.dma_start(
                        xis[t][:, :], x_d[128 * t : 128 * t + 128, :])
                xbf = xbp.tile([128, SIZE], BF16, tag="xbf")
                if t == 0:
                    # fill critical path: cast in two halves so the first
                    # four transposes start after the first 512 columns
                    nc.scalar.copy(xbf[:, 0:512], xis[t][:, 0:512])
                    nc.scalar.copy(xbf[:, 512:1024], xis[t][:, 512:1024])
                else:
                    nc.scalar.copy(xbf[:, :], xis[t][:, :])
                xTh = []
                for half in range(2):
                    ps = tps.tile([128, 4, 128], BF16, tag="tp")
                    for kk in range(4):
                        k = 4 * half + kk
                        nc.tensor.transpose(
                            ps[:, kk, :], xbf[:, 128 * k : 128 * k + 128],
                            identb[:, :])
                    xTa = xtp.tile([128, 4, 128], BF16, tag=f"xT{half}",
                                   name=f"xT{half}_{t}")
                    nc.vector.tensor_copy(xTa[:, :, :], ps[:, :, :])
                    xTh.append(xTa)
                xths[t] = xTh

            def emit_mms(t):
                """Matmuls + drain for tile t (xT halves already staged)."""
                xTh = xths.pop(t)
                ps0 = ops.tile([128, 512], F32, tag="mm0")
                ps1 = ops.tile([128, 512], F32, tag="mm1")
                ob = op.tile([128, SIZE], F32, tag="ob")
                if t == NTILES - 1:
                    # tail: k-major so the first half drains + DMAs while
                    # the second half's matmuls still run (the extra weight
                    # loads are hidden in the warm state)
                    for h, ps in ((0, ps0), (1, ps1)):
                        for k in range(NB):
                            nc.tensor.matmul(
                                ps[:, :], xTh[k // 4][:, k % 4, :],
                                ublk(k)[:, 512 * h : 512 * h + 512],
                                start=(k == 0), stop=(k == NB - 1))
                        if h == 0:
                            nc.scalar.copy(ob[:, 0:512], ps0[:, :])
                        else:
                            nc.vector.tensor_copy(ob[:, 512:1024], ps1[:, :])
                        nc.sync.dma_start(
                            out_d[128 * t : 128 * t + 128,
                                  512 * h : 512 * h + 512],
                            ob[:, 512 * h : 512 * h + 512])
                    return
                for k in range(NB):
                    xTk = xTh[k // 4][:, k % 4, :]
                    # same stationary (xT block) for both halves
                    nc.tensor.matmul(
                        ps0[:, :], xTk, ublk(k)[:, 0:512],
                        start=(k == 0), stop=(k == NB - 1))
                    nc.tensor.matmul(
                        ps1[:, :], xTk, ublk(k)[:, 512:1024],
                        start=(k == 0), stop=(k == NB - 1))
                # split drain across ACT and DVE
                nc.scalar.copy(ob[:, 0:512], ps0[:, :])
                nc.vector.tensor_copy(ob[:, 512:1024], ps1[:, :])
                nc.sync.dma_start(out_d[128 * t : 128 * t + 128, :], ob[:, :])

            # 2-deep software pipeline: tile t+2's load/cast/transposes are
            # emitted (= prioritized) ahead of tile t's matmuls + drain, so
            # the scheduler never starves the PE on a late cast
            emit_trans(0)
            emit_trans(1)
            for t in range(NTILES):
                if t + 2 < NTILES:
                    emit_trans(t + 2)
                emit_mms(t)

    nc.compile()
    return nc


def _get_program():
    if "nc" not in _CACHE:
        _CACHE["nc"] = _build_program()
    return _CACHE["nc"]


LAST_RESULTS = {}


def _host_u(dg, sb, sp, lg):
    """Compose U = (A_0 ... A_9)^T from the O(KB) params on the host.

    Returns [128, NB, SIZE] bf16 with u[p, k, j] = U[k*128 + p, j].
    """
    dg = dg.astype(np.float64)
    sb = sb.astype(np.float64)
    sp = sp.astype(np.float64)
    lg = lg.astype(np.float64)
    m = lg.max(axis=-1, keepdims=True)
    e = np.exp(lg - m)
    prob = e / e.sum(axis=-1, keepdims=True)          # (NT, MF)

    M = np.eye(SIZE)
    for i in range(NT - 1, -1, -1):
        dsum = prob[i] @ dg
        out = dsum[:, None] * M
        for f in range(MF):
            d = DIAG[f]
            c = prob[i, f]
            out[d:, :] += (c * sb[f, : SIZE - d])[:, None] * M[: SIZE - d, :]
            out[: SIZE - d, :] += (c * sp[f, : SIZE - d])[:, None] * M[d:, :]
        M = out
    U = M.T                                           # (SIZE, SIZE)
    u = U.reshape(NB, 128, SIZE).transpose(1, 0, 2)   # [p, k, j]
    return np.ascontiguousarray(u.astype(np.float32)).astype(BF16NP)


def kernel(input, diags, subdiags, superdiags, logit, _trace=False):
    from concourse.bass_utils import run_bass_kernel_spmd

    x = np.ascontiguousarray(np.asarray(input, dtype=np.float32))
    u = _host_u(
        np.asarray(diags, dtype=np.float32),
        np.asarray(subdiags, dtype=np.float32),
        np.asarray(superdiags, dtype=np.float32),
        np.asarray(logit, dtype=np.float32),
    )
    nc = _get_program()
    in_maps = [
        {"x": x[BPC * c : BPC * (c + 1)], "u": u}
        for c in range(N_CORES)
    ]
    res = run_bass_kernel_spmd(nc, in_maps, core_ids=list(range(N_CORES)),
                               trace=_trace)
    LAST_RESULTS["res"] = res
    out = np.concatenate([res.results[c]["out"] for c in range(N_CORES)], axis=0)
    return out
